# revision 23
# baseline (speedup 1.0000x reference)
"""Trainium2 Bass kernel for nn_Attention_58437325029549.

GQA attention layer: B=2, S=2048, D=2048, H=32 q-heads, KV=8 kv-heads, HD=64,
RoPE + causal softmax + o_proj, all fp32 I/O.

Sharding (8 NeuronCores): data-parallel over batch (2 groups of 4 cores),
tensor-parallel over kv-heads within each batch (each core owns 2 kv heads =
8 q heads = 512 of the 2048 o_proj contraction features). Each core computes a
full [S, D] partial o_proj output; the host sums the 4 partials per batch.

Per-core layout (feature-major): Q^T/K^T/V^T come out of one fused QKV
projection (x^T is the moving operand). Head pairing puts one kv0-head on
partitions 0-63 and one kv1-head on partitions 64-127 of each 128-partition
chunk so the K=64 QK^T matmuls of the two kv heads co-run in separate PE row
groups. Scores are built transposed [t2, t1] so softmax exp feeds the PV
matmul directly (no probs transpose); V is transposed to token-major via the
PE with an extra ones-column so the PV matmul also produces the softmax
denominator for free. Normalization is an outer-product broadcast of the
reciprocal denominator fused into the attn-output write.

Matmuls run in float32r (fp32 bits, PE rounds to 11-bit mantissa, 4x the
fp32 matmul throughput); PSUM accumulation stays fp32.
"""
import numpy as np

import concourse.bacc as bacc
import concourse.tile as tile
from concourse import mybir
from concourse import bass_utils

F32 = mybir.dt.float32
F32R = mybir.dt.float32r
AF = mybir.ActivationFunctionType

B, S, D = 2, 2048, 2048
H, KV, HD = 32, 8, 64
N_CORES = 8
NEG = -1e9

# matmul operand dtype: F32R (fast, ~1e-3 rel err) or F32 (exact, 4x slower)
DT = F32R


def emit(nc, tc, aps, dt):
    """Emit the per-core program. aps: dict of DRAM APs."""
    xt, wqkv, wot, cc2, ss2, tri, ident, y = (
        aps["xt"], aps["wqkv"], aps["wot"], aps["cc2"], aps["ss2"],
        aps["tri"], aps["ident"], aps["y"])

    with nc.allow_low_precision(reason="f32r operand staging is intentional; PSUM accumulation stays f32"):
        _emit(nc, tc, aps, dt)


def _emit(nc, tc, aps, dt):
    xt, wqkv, wot, cc2, ss2, tri, ident, y = (
        aps["xt"], aps["wqkv"], aps["wot"], aps["cc2"], aps["ss2"],
        aps["tri"], aps["ident"], aps["y"])

    with tc.tile_pool(name="persist", bufs=1) as pp, \
         tc.tile_pool(name="wkp", bufs=1) as wkp, \
         tc.tile_pool(name="xtp", bufs=5) as xtp, \
         tc.tile_pool(name="qtp", bufs=2) as qtp, \
         tc.tile_pool(name="ccssp", bufs=1) as ccssp, \
         tc.tile_pool(name="tmp", bufs=2) as tmpp, \
         tc.tile_pool(name="vtstp", bufs=2) as vtstp, \
         tc.tile_pool(name="attnp", bufs=2) as attnp, \
         tc.tile_pool(name="wobp", bufs=2) as wobp, \
         tc.tile_pool(name="expp", bufs=4) as expp, \
         tc.tile_pool(name="outp", bufs=3) as outp, \
         tc.tile_pool(name="rcp", bufs=2) as rcp, \
         tc.tile_pool(name="ps_s", bufs=2, space="PSUM") as ps_s, \
         tc.tile_pool(name="ps_pv", bufs=2, space="PSUM") as ps_pv, \
         tc.tile_pool(name="psA", bufs=2, space="PSUM") as psA:

        kt = pp.tile([128, S], dt, tag="kt", name="kt")
        vto = [[pp.tile([128, 65], dt, tag=f"v{j}_{c}", name=f"v{j}_{c}")
                for c in range(16)] for j in range(2)]
        tri_sb = pp.tile([128, 128], F32, tag="tri", name="tri_sb")
        id_sb = pp.tile([128, 128], dt, tag="id", name="id_sb")
        wk_t = [wkp.tile([128, 768], dt, tag=f"wk{k}", name=f"wk{k}")
                for k in range(16)]
        for k in range(16):
            nc.sync.dma_start(wk_t[k][:], wqkv[k * 128:(k + 1) * 128, :])

        # streamed per-q-tile state (python bookkeeping of rotating tiles)
        xt_tiles = {}      # n -> [16 tiles]
        q_tiles = {}       # n -> [4 tiles [128,512]]
        cs_tiles = {}      # n -> (cc, ss)
        attn_tiles = {}    # qt -> [4 tiles [128,512]]
        wob_tiles = {}     # (qt, dn) -> [4 tiles [128,512]]

        def emit_proj_group(n, m):
            ns = slice(n * 512, (n + 1) * 512)
            if m == 0:
                ts = []
                for g in range(4):
                    t = xtp.tile([128, 4, 512], dt, tag="xt", name=f"xt_{n}_{g}")
                    nc.sync.dma_start(
                        t[:], xt[g * 512:(g + 1) * 512, ns].rearrange(
                            "(c p) w -> p c w", p=128))
                    ts.append(t)
                xt_tiles[n] = ts
                if n == 0:
                    # constants needed later; issue after the hot-path loads
                    nc.gpsimd.dma_start(tri_sb[:], tri[:])
                    nc.gpsimd.dma_start(id_sb[:], ident[:])
                cc_sb = ccssp.tile([128, 512], F32, tag="cc", name=f"cc_{n}")
                ss_sb = ccssp.tile([128, 512], F32, tag="ss", name=f"ss_{n}")
                nc.sync.dma_start(cc_sb[:], cc2[:, ns])
                nc.sync.dma_start(ss_sb[:], ss2[:, ns])
                cs_tiles[n] = (cc_sb, ss_sb)
                q_tiles[n] = []
            cc_sb, ss_sb = cs_tiles[n]
            psum = psA.tile([128, 512], F32, tag="mm", name=f"p1_{n}_{m}")
            for k in range(16):
                nc.tensor.matmul(psum[:], wk_t[k][:, m * 128:(m + 1) * 128],
                                 xt_tiles[n][k // 4][:, k % 4, :], start=(k == 0), stop=(k == 15))
            if m < 5:
                # RoPE: dst = psum*cc + swap(psum)*ss
                qsw = tmpp.tile([128, 512], F32, tag="qsw", name=f"qsw_{n}_{m}")
                for (so, do) in ((32, 0), (0, 32), (96, 64), (64, 96)):
                    nc.vector.tensor_copy(qsw[do:do + 32, :], psum[so:so + 32, :])
                if m < 4:
                    dst = qtp.tile([128, 512], dt, tag=f"qt{m}", name=f"qt_{n}_{m}")
                    q_tiles[n].append(dst)
                    dstap = dst[:]
                else:
                    dstap = kt[:, ns]
                nc.vector.tensor_mul(dstap, psum[:], cc_sb[:])
                t2 = tmpp.tile([128, 512], F32, tag="t2", name=f"t2_{n}_{m}")
                nc.vector.tensor_mul(t2[:], qsw[:], ss_sb[:])
                nc.vector.tensor_add(dstap, dstap, t2[:])
            else:
                vtst = vtstp.tile([128, 512], dt, tag="vtst", name=f"vtst_{n}")
                nc.scalar.copy(vtst[:], psum[:])
                for j in (0, 1):
                    for q in range(4):
                        c = n * 4 + q
                        tr = psA.tile([128, 64], dt, tag="mm", name=f"tr_{j}_{c}",
                                      padded_shape=None)
                        nc.tensor.transpose(
                            tr[:], vtst[j * 64:(j + 1) * 64, q * 128:(q + 1) * 128],
                            id_sb[j * 64:(j + 1) * 64, j * 64:(j + 1) * 64])
                        nc.scalar.copy(vto[j][c][:, 0:64], tr[:])
                        nc.gpsimd.memset(vto[j][c][:, 64:65].bitcast(F32), 1.0)

        def emit_wob_load(qt_i, dn):
            t = wobp.tile([128, 4, 512], dt, tag="wob", name=f"wob_{qt_i}_{dn}")
            nc.sync.dma_start(
                t[:], wot[:, dn * 512:(dn + 1) * 512].rearrange(
                    "(c p) w -> p c w", p=128))
            wob_tiles[(qt_i, dn)] = t

        def emit_oproj(tm, dn):
            qt_i = tm // 4
            po = psA.tile([128, 512], F32, tag="mm", name=f"po_{tm}_{dn}")
            at = attn_tiles[qt_i]
            wb = wob_tiles[(qt_i, dn)]
            for r in range(4):
                nc.tensor.matmul(po[:], at[r][:, (tm % 4) * 128:(tm % 4 + 1) * 128],
                                 wb[:, r, :], start=(r == 0), stop=(r == 3))
            ob = outp.tile([128, 512], F32, tag="ob", name=f"ob_{tm}_{dn}")
            if dn % 2 == 0:
                nc.scalar.copy(ob[:], po[:])
            else:
                nc.vector.tensor_copy(ob[:], po[:])
            nc.sync.dma_start(y[tm * 128:(tm + 1) * 128,
                                dn * 512:(dn + 1) * 512], ob[:])

        # ---- prologue: project q-tile 0 ----
        for m in range(6):
            emit_proj_group(0, m)

        # ---- main pipeline over q-tiles ----
        for qt_i in range(4):
            q0 = qt_i * 512
            # work to interleave into this q-tile's chunk loop: heavy
            # projection groups are spread evenly across the whole tile so
            # they don't cluster and starve ACT of fresh scores.
            heavy = []
            light = []
            if qt_i < 3:
                heavy += [("proj", qt_i + 1, m) for m in range(6)]
            if qt_i >= 1:
                for dn in range(4):
                    light.append(("wob", qt_i - 1, dn))
                    light += [("oproj", 4 * (qt_i - 1) + t, dn) for t in range(4)]
            if qt_i == 3:
                light += [("wob", 3, dn) for dn in range(4)]
            nticks = (q0 // 128 + 4) * 4
            stride_h = max(2, nticks // (len(heavy) + 1)) if heavy else nticks + 1
            stride_l = max(1, nticks // (len(light) + 1)) if light else nticks + 1
            tick = 0

            def emit_item(it):
                kind, a, b = it
                if kind == "proj":
                    emit_proj_group(a, b)
                elif kind == "wob":
                    emit_wob_load(a, b)
                else:
                    emit_oproj(a, b)

            def maybe_work():
                if heavy and tick % stride_h == 0:
                    emit_item(heavy.pop(0))
                elif light and tick % stride_l == 0:
                    emit_item(light.pop(0))

            attn_tiles[qt_i] = [attnp.tile([128, 512], dt, tag=f"attn{r}",
                                           name=f"attn_{qt_i}_{r}")
                                for r in range(4)]
            for r in range(4):
                pvs = [ps_pv.tile([65, 512], F32, tag="pv",
                                  name=f"pv_{qt_i}_{r}_{j}") for j in (0, 1)]
                nch = q0 // 128 + 4
                for ci in range(nch):
                    c0 = ci * 128
                    st = max(c0 - q0, 0)
                    s = ps_s.tile([128, 1024], F32, tag="s",
                                  name=f"s_{qt_i}_{r}_{ci}")
                    for j in (0, 1):
                        kb = j * 64
                        base = j * 512
                        nc.tensor.matmul(
                            s[:, base + st:base + 512],
                            kt[kb:kb + 64, c0:c0 + 128],
                            q_tiles[qt_i][r][kb:kb + 64, st:512],
                            start=True, stop=True)
                    if c0 >= q0:
                        for j in (0, 1):
                            base = j * 512
                            nc.vector.tensor_add(
                                s[:, base + st:base + st + 128],
                                s[:, base + st:base + st + 128], tri_sb[:])
                    e = expp.tile([128, 1024], dt, tag="e",
                                  name=f"e_{qt_i}_{r}_{ci}")
                    if st == 0:
                        nc.scalar.activation(e[:], s[:], AF.Exp)
                    else:
                        for j in (0, 1):
                            base = j * 512
                            nc.gpsimd.memset(e[:, base:base + st].bitcast(F32), 0.0)
                            nc.scalar.activation(e[:, base + st:base + 512],
                                                 s[:, base + st:base + 512],
                                                 AF.Exp)
                    for j in (0, 1):
                        nc.tensor.matmul(
                            pvs[j][:], vto[j][ci][:, 0:65],
                            e[:, j * 512:j * 512 + 512],
                            start=(ci == 0), stop=(ci == nch - 1))
                    tick += 1
                    maybe_work()
                for j in (0, 1):
                    kb = j * 64
                    rc = rcp.tile([1, 512], F32, tag="rc", name=f"rc_{qt_i}_{r}_{j}")
                    nc.vector.reciprocal(rc[:], pvs[j][64:65, :])
                    rcb = rcp.tile([64, 512], F32, tag="rcb", name=f"rcb_{qt_i}_{r}_{j}")
                    nc.gpsimd.partition_broadcast(rcb[:], rc[:])
                    nc.vector.tensor_mul(attn_tiles[qt_i][r][kb:kb + 64, :],
                                         pvs[j][0:64, :], rcb[:])
            # drain any leftover interleave work for this q-tile
            for it in heavy + light:
                emit_item(it)
            heavy, light = [], []

        # ---- tail: o_proj for the last q-tile (wob prefetched in-loop) ----
        for dn in range(4):
            for t in range(4):
                emit_oproj(12 + t, dn)


def build_nc(dt=DT, reps=1):
    nc = bacc.Bacc("TRN2", target_bir_lowering=False, debug=False,
                   num_devices=N_CORES)
    aps = {
        "xt": nc.dram_tensor("xt", [D, S], dt, kind="ExternalInput").ap(),
        "wqkv": nc.dram_tensor("wqkv", [D, 768], dt, kind="ExternalInput").ap(),
        "wot": nc.dram_tensor("wot", [512, D], dt, kind="ExternalInput").ap(),
        "cc2": nc.dram_tensor("cc2", [128, S], F32, kind="ExternalInput").ap(),
        "ss2": nc.dram_tensor("ss2", [128, S], F32, kind="ExternalInput").ap(),
        "tri": nc.dram_tensor("tri", [128, 128], F32, kind="ExternalInput").ap(),
        "ident": nc.dram_tensor("ident", [128, 128], dt, kind="ExternalInput").ap(),
        "y": nc.dram_tensor("y", [S, D], F32, kind="ExternalOutput").ap(),
    }
    with tile.TileContext(nc) as tc:
        if reps == 1:
            emit(nc, tc, aps, dt)
        else:
            with tc.For_i(0, reps, 1):
                emit(nc, tc, aps, dt)
    nc.compile()
    return nc


def make_in_maps(x, cos, sin, wq, wk, wv, wo):
    """Host-side shard + layout prep. Returns list of 8 per-core input dicts."""
    x = np.asarray(x, np.float32)
    cos, sin = np.asarray(cos, np.float32), np.asarray(sin, np.float32)
    wq, wk, wv, wo = (np.asarray(a, np.float32) for a in (wq, wk, wv, wo))

    p = np.arange(128)
    cc2 = np.ascontiguousarray(cos[:, p % 32].T)                       # [128, S]
    sgn = np.where((p % 64) < 32, -1.0, 1.0).astype(np.float32)
    ss2 = np.ascontiguousarray(sin[:, p % 32].T * sgn[:, None])        # [128, S]
    u = np.arange(128)
    tri = np.where(u[:, None] <= u[None, :], 0.0, NEG).astype(np.float32)
    ident = np.eye(128, dtype=np.float32)

    scale = 1.0 / np.sqrt(HD)
    in_maps = []
    for c in range(N_CORES):
        b, g = divmod(c, 4)
        head_rows = []
        for r in range(4):
            for h in (8 * g + r, 8 * g + 4 + r):
                head_rows.append(wq[h * 64:(h + 1) * 64] * scale)
        wq_g = np.concatenate(head_rows, 0)                            # [512, D]
        wk_g = wk[(2 * g) * 64:(2 * g + 2) * 64]                       # [128, D]
        wv_g = wv[(2 * g) * 64:(2 * g + 2) * 64]                       # [128, D]
        wqkv_g = np.ascontiguousarray(
            np.concatenate([wq_g, wk_g, wv_g], 0).T)                   # [D, 768]
        wo_cols = []
        for r in range(4):
            for h in (8 * g + r, 8 * g + 4 + r):
                wo_cols.append(wo[:, h * 64:(h + 1) * 64])
        wot_g = np.ascontiguousarray(np.concatenate(wo_cols, 1).T)     # [512, D]
        xt_b = np.ascontiguousarray(x[b].T)                            # [D, S]
        in_maps.append({"xt": xt_b, "wqkv": wqkv_g, "wot": wot_g,
                        "cc2": cc2, "ss2": ss2, "tri": tri, "ident": ident})
    return in_maps


_NC_CACHE = {}


def kernel(x, cos, sin, mask, wq, wk, wv, wo):
    """Full-input attention kernel distributed over 8 NeuronCores."""
    key = ("main", DT, 1)
    if key not in _NC_CACHE:
        _NC_CACHE[key] = build_nc(DT, 1)
    nc = _NC_CACHE[key]
    in_maps = make_in_maps(x, cos, sin, wq, wk, wv, wo)
    res = bass_utils.run_bass_kernel_spmd(nc, in_maps, core_ids=list(range(N_CORES)))
    out = np.zeros((B, S, D), np.float32)
    for c in range(N_CORES):
        out[c // 4] += res.results[c]["y"]
    return out
